# revision 8
# baseline (speedup 1.0000x reference)
"""Trainium2 Bass kernel for nn_MeshTransformer (8-core SPMD, V-sharded).

Computes, for each of BS=256 (b,s) pairs:
    out[bs, v, i] = sum_{p,j} ws[bs,p] * R[i,j](bs,p) * deformed[p,v,j]
                    + sum_p w[bs,p] * t[bs,p,i]
with R the XYZ-euler rotation, ws = w * scale, deformed = base + offsets.

Mapping:
  - Vertex dim V (2562, padded to 2568) is sharded 8 ways (321/core).
  - Each core computes all 256 weight matrices W_i = [K=128|64+64, M=bs]
    on-chip (trig on ACT, products on DVE/GPSIMD) and contracts them
    against its deformed slice on the PE (float32r matmuls, N=321).
  - lhsT partition layout packs rotation column j in 64-partition blocks:
      LA_i = [W_i0 (p 0..63) ; W_i1 (p 64..127)]
      LB_i = [W_i2 (p 0..63) ; wt_i (p 64..127)]  paired with rhs
      DA   = [deformed_0    ; deformed_1      ]
      DB   = [deformed_2    ; ones            ]  (ones row folds translation)
  - PSUM drained into an interleaved [bs, (v,3)] SBUF tile so the output
    DMA is fully contiguous per partition row.
"""

import numpy as np
from contextlib import ExitStack

import concourse.bass as bass
import concourse.tile as tile
from concourse import bacc, mybir
from concourse.bass_utils import run_bass_kernel_spmd

B, S, P, V = 16, 16, 64, 2562
BS = B * S              # 256
N_CORES = 8
VPAD = 2568             # multiple of 8
VC = VPAD // N_CORES    # 321 vertices per core

F32 = mybir.dt.float32
F32R = mybir.dt.float32r
AF = mybir.ActivationFunctionType
ALU = mybir.AluOpType
HALF_PI = float(np.pi / 2)


def _build_kernel():
    nc = bacc.Bacc("TRN2", target_bir_lowering=False, debug=False)

    ang_d = nc.dram_tensor("ang", [128, 3 * BS], F32, kind="ExternalInput").ap()
    wraw_d = nc.dram_tensor("wraw", [128, BS], F32, kind="ExternalInput").ap()
    scl_d = nc.dram_tensor("scl", [128, BS], F32, kind="ExternalInput").ap()
    trn_d = nc.dram_tensor("trn", [64, 3 * BS], F32, kind="ExternalInput").ap()
    offt_d = nc.dram_tensor("offt", [192, VC], F32, kind="ExternalInput").ap()
    bset_d = nc.dram_tensor("bset", [192, VC], F32, kind="ExternalInput").ap()
    out_d = nc.dram_tensor("out", [BS, VC * 3], F32, kind="ExternalOutput").ap()

    with tile.TileContext(nc) as tc, ExitStack() as ctx:
        pool = ctx.enter_context(tc.tile_pool(name="work", bufs=1))
        psum = ctx.enter_context(tc.tile_pool(name="psum", bufs=3, space="PSUM"))

        # ---- input tiles ----
        ang = pool.tile([128, 3 * BS], F32, tag="ang")
        wraw = pool.tile([128, BS], F32, tag="wraw")
        scl = pool.tile([128, BS], F32, tag="scl")
        trn = pool.tile([128, 3 * BS], F32, tag="trn")   # data in partitions 64:128
        da = pool.tile([128, VC], F32, tag="da")
        db = pool.tile([128, VC], F32, tag="db")
        offb = pool.tile([64, VC], F32, tag="offb")
        bsa = pool.tile([128, VC], F32, tag="bsa")
        bsb = pool.tile([64, VC], F32, tag="bsb")

        nc.sync.dma_start(out=ang[:], in_=ang_d[:])
        nc.sync.dma_start(out=wraw[:], in_=wraw_d[:])
        nc.sync.dma_start(out=scl[:], in_=scl_d[:])
        nc.sync.dma_start(out=trn[64:128, :], in_=trn_d[:])
        nc.sync.dma_start(out=bsa[:], in_=bset_d[0:128, :])
        nc.sync.dma_start(out=bsb[:], in_=bset_d[128:192, :])
        nc.sync.dma_start(out=da[:], in_=offt_d[0:128, :])
        nc.sync.dma_start(out=offb[:], in_=offt_d[128:192, :])

        # ---- deformed (rhs) ----
        nc.gpsimd.tensor_add(da[:], da[:], bsa[:])            # deformed j=0,1
        nc.gpsimd.tensor_add(db[0:64, :], offb[:], bsb[:])    # deformed j=2
        nc.vector.memset(db[64:128, :], 1.0)                  # ones (translation)

        # ---- trig ----
        sin = pool.tile([128, 3 * BS], F32, tag="sin")
        cos = pool.tile([128, 3 * BS], F32, tag="cos")
        # Sin spline domain is [-pi, pi] but |angle| can reach ~4.8. Half-angle
        # identity keeps every Sin argument in-domain (|x/2| < pi):
        #   sh = sin(x/2), ch = cos(x/2) = sin(pi/2 - |x|/2)
        #   sin(x) = 2*sh*ch,  cos(x) = 1 - 2*sh^2
        sh = pool.tile([128, 3 * BS], F32, tag="sh")
        ch = pool.tile([128, 3 * BS], F32, tag="ch")
        axa = pool.tile([128, 3 * BS], F32, tag="axa")
        sh2 = pool.tile([128, 3 * BS], F32, tag="sh2")
        halfpi = pool.tile([128, 1], F32, tag="halfpi")
        nc.vector.memset(halfpi[:], HALF_PI)
        nc.scalar.activation(sh[:], ang[:], AF.Sin, scale=0.5)
        nc.scalar.activation(axa[:], ang[:], AF.Abs)
        nc.scalar.activation(ch[:], axa[:], AF.Sin, bias=halfpi[:], scale=-0.5)
        nc.vector.scalar_tensor_tensor(
            sin[:], sh[:], 2.0, ch[:], op0=ALU.mult, op1=ALU.mult)
        nc.gpsimd.tensor_mul(sh2[:], sh[:], sh[:])
        nc.vector.tensor_scalar(cos[:], sh2[:], -2.0, 1.0,
                                op0=ALU.mult, op1=ALU.add)
        sa = sin[:, 0:BS]; sb = sin[:, BS:2 * BS]; sc = sin[:, 2 * BS:3 * BS]
        ca = cos[:, 0:BS]; cb = cos[:, BS:2 * BS]; cc = cos[:, 2 * BS:3 * BS]

        # ---- shared products ----
        ws = pool.tile([128, BS], F32, tag="ws")
        wca = pool.tile([128, BS], F32, tag="wca")
        wsa = pool.tile([128, BS], F32, tag="wsa")
        cbcc = pool.tile([128, BS], F32, tag="cbcc")
        cbsc = pool.tile([128, BS], F32, tag="cbsc")
        sbcc = pool.tile([128, BS], F32, tag="sbcc")
        sbsc = pool.tile([128, BS], F32, tag="sbsc")

        nc.vector.tensor_mul(ws[:], wraw[:], scl[:])
        nc.vector.tensor_mul(wca[:], ws[:], ca)
        nc.vector.tensor_mul(wsa[:], ws[:], sa)
        nc.gpsimd.tensor_mul(cbcc[:], cb, cc)
        nc.gpsimd.tensor_mul(cbsc[:], cb, sc)
        nc.vector.tensor_mul(sbcc[:], sb, cc)
        nc.vector.tensor_mul(sbsc[:], sb, sc)

        # ---- weight matrices (lhsT tiles) ----
        la = [pool.tile([128, BS], F32, name=f"la{i}", tag=f"la{i}") for i in range(3)]
        lb = [pool.tile([128, BS], F32, name=f"lb{i}", tag=f"lb{i}") for i in range(3)]
        t1 = pool.tile([64, BS], F32, tag="t1")
        t2 = pool.tile([64, BS], F32, tag="t2")
        t3 = pool.tile([64, BS], F32, tag="t3")
        t4 = pool.tile([64, BS], F32, tag="t4")

        lo = slice(0, 64)
        hi = slice(64, 128)

        # row i=0:  [ws*cbcc ; -ws*cbsc]  /  [ws*sb ; wt_0]
        nc.gpsimd.tensor_mul(la[0][lo, :], ws[lo, :], cbcc[lo, :])
        nc.vector.scalar_tensor_tensor(
            la[0][hi, :], cbsc[hi, :], -1.0, ws[hi, :], op0=ALU.mult, op1=ALU.mult)
        nc.vector.tensor_mul(lb[0][lo, :], ws[lo, :], sb[lo, :])
        nc.gpsimd.tensor_mul(lb[0][hi, :], wraw[hi, :], trn[hi, 0:BS])

        # row i=1:  [wca*sc + wsa*sbcc ; wca*cc - wsa*sbsc] / [-wsa*cb ; wt_1]
        nc.vector.tensor_mul(t1[:], wca[lo, :], sc[lo, :])
        nc.vector.tensor_mul(t2[:], wsa[lo, :], sbcc[lo, :])
        nc.vector.tensor_add(la[1][lo, :], t1[:], t2[:])
        nc.vector.tensor_mul(t3[:], wca[hi, :], cc[hi, :])
        nc.vector.tensor_mul(t4[:], wsa[hi, :], sbsc[hi, :])
        nc.vector.tensor_sub(la[1][hi, :], t3[:], t4[:])
        nc.vector.scalar_tensor_tensor(
            lb[1][lo, :], cb[lo, :], -1.0, wsa[lo, :], op0=ALU.mult, op1=ALU.mult)
        nc.gpsimd.tensor_mul(lb[1][hi, :], wraw[hi, :], trn[hi, BS:2 * BS])

        # row i=2:  [wsa*sc - wca*sbcc ; wsa*cc + wca*sbsc] / [wca*cb ; wt_2]
        t5 = pool.tile([64, BS], F32, tag="t5")
        t6 = pool.tile([64, BS], F32, tag="t6")
        t7 = pool.tile([64, BS], F32, tag="t7")
        t8 = pool.tile([64, BS], F32, tag="t8")
        nc.gpsimd.tensor_mul(t5[:], wsa[lo, :], sc[lo, :])
        nc.gpsimd.tensor_mul(t6[:], wca[lo, :], sbcc[lo, :])
        nc.gpsimd.tensor_sub(la[2][lo, :], t5[:], t6[:])
        nc.vector.tensor_mul(t7[:], wsa[hi, :], cc[hi, :])
        nc.vector.tensor_mul(t8[:], wca[hi, :], sbsc[hi, :])
        nc.vector.tensor_add(la[2][hi, :], t7[:], t8[:])
        nc.gpsimd.tensor_mul(lb[2][lo, :], wca[lo, :], cb[lo, :])
        nc.gpsimd.tensor_mul(lb[2][hi, :], wraw[hi, :], trn[hi, 2 * BS:3 * BS])

        # ---- matmuls + drain + output ----
        for h in range(2):
            osb = pool.tile([128, VC, 3], F32, tag=f"osb{h}")
            ms = slice(h * 128, (h + 1) * 128)
            for i in range(3):
                ps = psum.tile([128, VC], F32)
                nc.tensor.matmul(
                    ps[:], la[i][:, ms], da[:],
                    start=True, stop=False)
                nc.tensor.matmul(
                    ps[:], lb[i][:, ms], db[:],
                    start=False, stop=True)
                if i % 2 == 0:
                    nc.vector.tensor_copy(osb[:, :, i], ps[:])
                else:
                    nc.scalar.copy(osb[:, :, i], ps[:])
            nc.sync.dma_start(out=out_d[ms, :], in_=osb[:].rearrange("p a b -> p (a b)"))

    nc.compile()
    return nc


_NC_CACHE = None


def _get_nc():
    global _NC_CACHE
    if _NC_CACHE is None:
        _NC_CACHE = _build_kernel()
    return _NC_CACHE


def _prep_inputs(scales, transforms, prototype_weights, prototype_offsets, base_verts):
    """Host-side shard/layout prep (layout only, no reference math)."""
    f = np.float32
    scl1 = np.asarray(scales, f).reshape(BS)
    tf = np.asarray(transforms, f).reshape(BS, P, 6)

    ang_h = np.ascontiguousarray(tf[:, :, 3:6].transpose(1, 2, 0).reshape(P, 3 * BS))
    ang = np.concatenate([ang_h, ang_h], axis=0)                  # [128, 768]
    w_h = np.ascontiguousarray(np.asarray(prototype_weights, f).reshape(BS, P).T)
    wraw = np.concatenate([w_h, w_h], axis=0)                     # [128, 256]
    scl = np.ascontiguousarray(np.broadcast_to(scl1[None, :], (128, BS)))
    trn = np.ascontiguousarray(tf[:, :, 0:3].transpose(1, 2, 0).reshape(P, 3 * BS))

    offp = np.zeros((P, VPAD, 3), f)
    offp[:, :V] = np.asarray(prototype_offsets, f)
    offt = np.ascontiguousarray(offp.transpose(2, 0, 1).reshape(192, VPAD))
    basep = np.zeros((VPAD, 3), f)
    basep[:V] = np.asarray(base_verts, f)
    bset = np.ascontiguousarray(
        np.broadcast_to(basep.T[:, None, :], (3, P, VPAD)).reshape(192, VPAD))

    in_maps = []
    for core in range(N_CORES):
        vs = slice(core * VC, (core + 1) * VC)
        in_maps.append({
            "ang": ang,
            "wraw": wraw,
            "scl": scl,
            "trn": trn,
            "offt": np.ascontiguousarray(offt[:, vs]),
            "bset": np.ascontiguousarray(bset[:, vs]),
        })
    return in_maps


def kernel(scales, transforms, prototype_weights, prototype_offsets, base_verts):
    nc = _get_nc()
    in_maps = _prep_inputs(
        scales, transforms, prototype_weights, prototype_offsets, base_verts)
    res = run_bass_kernel_spmd(nc, in_maps, list(range(N_CORES)))
    parts = [res.results[c]["out"].reshape(BS, VC, 3) for c in range(N_CORES)]
    full = np.concatenate(parts, axis=1)[:, :V, :]
    return np.ascontiguousarray(full.astype(np.float32))


# revision 10
# speedup vs baseline: 1.2059x; 1.2059x over previous
"""Trainium2 Bass kernel for nn_MeshTransformer (8-core SPMD, V-sharded).

Computes, for each of BS=256 (b,s) pairs:
    out[bs, v, i] = sum_{p,j} ws[bs,p] * R[i,j](bs,p) * deformed[p,v,j]
                    + sum_p w[bs,p] * t[bs,p,i]
with R the XYZ-euler rotation, ws = w * scale, deformed = base + offsets.

Mapping:
  - Vertex dim V (2562, padded to 2568) is sharded 8 ways (321/core).
  - Each core computes all 256 weight matrices W_i = [K=128|64+64, M=bs]
    on-chip (trig on ACT, products on DVE/GPSIMD) and contracts them
    against its deformed slice on the PE (float32r matmuls, N=321).
  - lhsT partition layout packs rotation column j in 64-partition blocks:
      LA_i = [W_i0 (p 0..63) ; W_i1 (p 64..127)]
      LB_i = [W_i2 (p 0..63) ; wt_i (p 64..127)]  paired with rhs
      DA   = [deformed_0    ; deformed_1      ]
      DB   = [deformed_2    ; ones            ]  (ones row folds translation)
  - PSUM drained into an interleaved [bs, (v,3)] SBUF tile so the output
    DMA is fully contiguous per partition row.
"""

import numpy as np
from contextlib import ExitStack

import concourse.bass as bass
import concourse.tile as tile
from concourse import bacc, mybir
from concourse.bass_utils import run_bass_kernel_spmd

B, S, P, V = 16, 16, 64, 2562
BS = B * S              # 256
N_CORES = 8
VPAD = 2576             # multiple of 8; per-core N must be even for f32r matmul
VC = VPAD // N_CORES    # 321 vertices per core

F32 = mybir.dt.float32
F32R = mybir.dt.float32r
AF = mybir.ActivationFunctionType
ALU = mybir.AluOpType
HALF_PI = float(np.pi / 2)


def _build_kernel():
    nc = bacc.Bacc("TRN2", target_bir_lowering=False, debug=False)

    ang_d = nc.dram_tensor("ang", [128, 3 * BS], F32, kind="ExternalInput").ap()
    wraw_d = nc.dram_tensor("wraw", [128, BS], F32, kind="ExternalInput").ap()
    scl_d = nc.dram_tensor("scl", [128, BS], F32, kind="ExternalInput").ap()
    trn_d = nc.dram_tensor("trn", [64, 3 * BS], F32, kind="ExternalInput").ap()
    offt_d = nc.dram_tensor("offt", [192, VC], F32R, kind="ExternalInput").ap()
    bset_d = nc.dram_tensor("bset", [192, VC], F32R, kind="ExternalInput").ap()
    ones_d = nc.dram_tensor("ones", [64, VC], F32R, kind="ExternalInput").ap()
    out_d = nc.dram_tensor("out", [BS, VC * 3], F32, kind="ExternalOutput").ap()

    with tile.TileContext(nc) as tc, ExitStack() as ctx:
        pool = ctx.enter_context(tc.tile_pool(name="work", bufs=1))
        psum = ctx.enter_context(tc.tile_pool(name="psum", bufs=3, space="PSUM"))

        # ---- input tiles ----
        ang = pool.tile([128, 3 * BS], F32, tag="ang")
        wraw = pool.tile([128, BS], F32, tag="wraw")
        scl = pool.tile([128, BS], F32, tag="scl")
        trn = pool.tile([128, 3 * BS], F32, tag="trn")   # data in partitions 64:128
        da = pool.tile([128, VC], F32R, tag="da")
        db = pool.tile([128, VC], F32R, tag="db")
        offb = pool.tile([64, VC], F32R, tag="offb")
        bsa = pool.tile([128, VC], F32R, tag="bsa")
        bsb = pool.tile([64, VC], F32R, tag="bsb")

        nc.sync.dma_start(out=ang[:], in_=ang_d[:])
        nc.sync.dma_start(out=wraw[:], in_=wraw_d[:])
        nc.sync.dma_start(out=scl[:], in_=scl_d[:])
        nc.sync.dma_start(out=trn[64:128, :], in_=trn_d[:])
        nc.sync.dma_start(out=bsa[:], in_=bset_d[0:128, :])
        nc.sync.dma_start(out=bsb[:], in_=bset_d[128:192, :])
        nc.sync.dma_start(out=da[:], in_=offt_d[0:128, :])
        nc.sync.dma_start(out=offb[:], in_=offt_d[128:192, :])

        # ---- deformed (rhs) ----
        nc.gpsimd.tensor_add(da[:], da[:], bsa[:])            # deformed j=0,1
        nc.gpsimd.tensor_add(db[0:64, :], offb[:], bsb[:])    # deformed j=2
        nc.sync.dma_start(out=db[64:128, :], in_=ones_d[:])   # ones (translation)

        # ---- trig ----
        sin = pool.tile([128, 3 * BS], F32, tag="sin")
        cos = pool.tile([128, 3 * BS], F32, tag="cos")
        # Sin spline domain is [-pi, pi] but |angle| can reach ~4.8. Half-angle
        # identity keeps every Sin argument in-domain (|x/2| < pi):
        #   sh = sin(x/2), ch = cos(x/2) = sin(pi/2 - |x|/2)
        #   sin(x) = 2*sh*ch,  cos(x) = 1 - 2*sh^2
        sh = pool.tile([128, 3 * BS], F32, tag="sh")
        ch = pool.tile([128, 3 * BS], F32, tag="ch")
        axa = pool.tile([128, 3 * BS], F32, tag="axa")
        sh2 = pool.tile([128, 3 * BS], F32, tag="sh2")
        halfpi = pool.tile([128, 1], F32, tag="halfpi")
        nc.vector.memset(halfpi[:], HALF_PI)
        nc.scalar.activation(sh[:], ang[:], AF.Sin, scale=0.5)
        nc.scalar.activation(axa[:], ang[:], AF.Abs)
        nc.scalar.activation(ch[:], axa[:], AF.Sin, bias=halfpi[:], scale=-0.5)
        nc.vector.scalar_tensor_tensor(
            sin[:], sh[:], 2.0, ch[:], op0=ALU.mult, op1=ALU.mult)
        nc.gpsimd.tensor_mul(sh2[:], sh[:], sh[:])
        nc.vector.tensor_scalar(cos[:], sh2[:], -2.0, 1.0,
                                op0=ALU.mult, op1=ALU.add)
        sa = sin[:, 0:BS]; sb = sin[:, BS:2 * BS]; sc = sin[:, 2 * BS:3 * BS]
        ca = cos[:, 0:BS]; cb = cos[:, BS:2 * BS]; cc = cos[:, 2 * BS:3 * BS]

        # ---- shared products ----
        ws = pool.tile([128, BS], F32, tag="ws")
        wca = pool.tile([128, BS], F32, tag="wca")
        wsa = pool.tile([128, BS], F32, tag="wsa")
        cbcc = pool.tile([128, BS], F32, tag="cbcc")
        cbsc = pool.tile([128, BS], F32, tag="cbsc")
        sbcc = pool.tile([128, BS], F32, tag="sbcc")
        sbsc = pool.tile([128, BS], F32, tag="sbsc")

        nc.vector.tensor_mul(ws[:], wraw[:], scl[:])
        nc.vector.tensor_mul(wca[:], ws[:], ca)
        nc.vector.tensor_mul(wsa[:], ws[:], sa)
        nc.gpsimd.tensor_mul(cbcc[:], cb, cc)
        nc.gpsimd.tensor_mul(cbsc[:], cb, sc)
        nc.vector.tensor_mul(sbcc[:], sb, cc)
        nc.vector.tensor_mul(sbsc[:], sb, sc)

        # ---- weight matrices (lhsT tiles) ----
        la = [pool.tile([128, BS], F32R, name=f"la{i}", tag=f"la{i}") for i in range(3)]
        lb = [pool.tile([128, BS], F32R, name=f"lb{i}", tag=f"lb{i}") for i in range(3)]
        t1 = pool.tile([64, BS], F32, tag="t1")
        t2 = pool.tile([64, BS], F32, tag="t2")
        t3 = pool.tile([64, BS], F32, tag="t3")
        t4 = pool.tile([64, BS], F32, tag="t4")

        lo = slice(0, 64)
        hi = slice(64, 128)

        # row i=0:  [ws*cbcc ; -ws*cbsc]  /  [ws*sb ; wt_0]
        nc.gpsimd.tensor_mul(la[0][lo, :], ws[lo, :], cbcc[lo, :])
        nc.vector.scalar_tensor_tensor(
            la[0][hi, :], cbsc[hi, :], -1.0, ws[hi, :], op0=ALU.mult, op1=ALU.mult)
        nc.vector.tensor_mul(lb[0][lo, :], ws[lo, :], sb[lo, :])
        nc.gpsimd.tensor_mul(lb[0][hi, :], wraw[hi, :], trn[hi, 0:BS])

        # row i=1:  [wca*sc + wsa*sbcc ; wca*cc - wsa*sbsc] / [-wsa*cb ; wt_1]
        nc.vector.tensor_mul(t1[:], wca[lo, :], sc[lo, :])
        nc.vector.tensor_mul(t2[:], wsa[lo, :], sbcc[lo, :])
        nc.vector.tensor_add(la[1][lo, :], t1[:], t2[:])
        nc.vector.tensor_mul(t3[:], wca[hi, :], cc[hi, :])
        nc.vector.tensor_mul(t4[:], wsa[hi, :], sbsc[hi, :])
        nc.vector.tensor_sub(la[1][hi, :], t3[:], t4[:])
        nc.vector.scalar_tensor_tensor(
            lb[1][lo, :], cb[lo, :], -1.0, wsa[lo, :], op0=ALU.mult, op1=ALU.mult)
        nc.gpsimd.tensor_mul(lb[1][hi, :], wraw[hi, :], trn[hi, BS:2 * BS])

        # row i=2:  [wsa*sc - wca*sbcc ; wsa*cc + wca*sbsc] / [wca*cb ; wt_2]
        t5 = pool.tile([64, BS], F32, tag="t5")
        t6 = pool.tile([64, BS], F32, tag="t6")
        t7 = pool.tile([64, BS], F32, tag="t7")
        t8 = pool.tile([64, BS], F32, tag="t8")
        nc.gpsimd.tensor_mul(t5[:], wsa[lo, :], sc[lo, :])
        nc.gpsimd.tensor_mul(t6[:], wca[lo, :], sbcc[lo, :])
        nc.gpsimd.tensor_sub(la[2][lo, :], t5[:], t6[:])
        nc.vector.tensor_mul(t7[:], wsa[hi, :], cc[hi, :])
        nc.vector.tensor_mul(t8[:], wca[hi, :], sbsc[hi, :])
        nc.vector.tensor_add(la[2][hi, :], t7[:], t8[:])
        nc.gpsimd.tensor_mul(lb[2][lo, :], wca[lo, :], cb[lo, :])
        nc.gpsimd.tensor_mul(lb[2][hi, :], wraw[hi, :], trn[hi, 2 * BS:3 * BS])

        # ---- matmuls + drain + output ----
        for h in range(2):
            osb = pool.tile([128, VC, 3], F32, tag=f"osb{h}")
            ms = slice(h * 128, (h + 1) * 128)
            for i in range(3):
                ps = psum.tile([128, VC], F32)
                nc.tensor.matmul(
                    ps[:], la[i][:, ms], da[:],
                    start=True, stop=False)
                nc.tensor.matmul(
                    ps[:], lb[i][:, ms], db[:],
                    start=False, stop=True)
                if i % 2 == 0:
                    nc.vector.tensor_copy(osb[:, :, i], ps[:])
                else:
                    nc.scalar.copy(osb[:, :, i], ps[:])
            nc.sync.dma_start(out=out_d[ms, :], in_=osb[:].rearrange("p a b -> p (a b)"))

    nc.compile()
    return nc


_NC_CACHE = None


def _get_nc():
    global _NC_CACHE
    if _NC_CACHE is None:
        _NC_CACHE = _build_kernel()
    return _NC_CACHE


def _prep_inputs(scales, transforms, prototype_weights, prototype_offsets, base_verts):
    """Host-side shard/layout prep (layout only, no reference math)."""
    f = np.float32
    scl1 = np.asarray(scales, f).reshape(BS)
    tf = np.asarray(transforms, f).reshape(BS, P, 6)

    ang_h = np.ascontiguousarray(tf[:, :, 3:6].transpose(1, 2, 0).reshape(P, 3 * BS))
    ang = np.concatenate([ang_h, ang_h], axis=0)                  # [128, 768]
    w_h = np.ascontiguousarray(np.asarray(prototype_weights, f).reshape(BS, P).T)
    wraw = np.concatenate([w_h, w_h], axis=0)                     # [128, 256]
    scl = np.ascontiguousarray(np.broadcast_to(scl1[None, :], (128, BS)))
    trn = np.ascontiguousarray(tf[:, :, 0:3].transpose(1, 2, 0).reshape(P, 3 * BS))

    offp = np.zeros((P, VPAD, 3), f)
    offp[:, :V] = np.asarray(prototype_offsets, f)
    offt = np.ascontiguousarray(offp.transpose(2, 0, 1).reshape(192, VPAD))
    basep = np.zeros((VPAD, 3), f)
    basep[:V] = np.asarray(base_verts, f)
    bset = np.ascontiguousarray(
        np.broadcast_to(basep.T[:, None, :], (3, P, VPAD)).reshape(192, VPAD))

    in_maps = []
    for core in range(N_CORES):
        vs = slice(core * VC, (core + 1) * VC)
        in_maps.append({
            "ang": ang,
            "wraw": wraw,
            "scl": scl,
            "trn": trn,
            "offt": np.ascontiguousarray(offt[:, vs]),
            "bset": np.ascontiguousarray(bset[:, vs]),
            "ones": np.ones((64, VC), f),
        })
    return in_maps


def kernel(scales, transforms, prototype_weights, prototype_offsets, base_verts):
    nc = _get_nc()
    in_maps = _prep_inputs(
        scales, transforms, prototype_weights, prototype_offsets, base_verts)
    res = run_bass_kernel_spmd(nc, in_maps, list(range(N_CORES)))
    parts = [res.results[c]["out"].reshape(BS, VC, 3) for c in range(N_CORES)]
    full = np.concatenate(parts, axis=1)[:, :V, :]
    return np.ascontiguousarray(full.astype(np.float32))


# revision 12
# speedup vs baseline: 1.4697x; 1.2187x over previous
"""Trainium2 Bass kernel for nn_MeshTransformer (8-core SPMD, V-sharded).

Computes, for each of BS=256 (b,s) pairs:
    out[bs, v, i] = sum_{p,j} ws[bs,p] * R[i,j](bs,p) * deformed[p,v,j]
                    + sum_p w[bs,p] * t[bs,p,i]
with R the XYZ-euler rotation, ws = w * scale, deformed = base + offsets.

Mapping:
  - Vertex dim V (2562, padded to 2576) is sharded 8 ways (322/core).
  - Each core computes all 256 weight matrices on-chip (trig on ACT via
    half-angle identities, products on DVE/GPSIMD in fp16) and contracts
    them against its deformed slice on the PE (fp16 matmuls, fp32 PSUM).
  - lhsT partition layout packs rotation column j in 64-partition blocks,
    paired with a stacked rhs:
      LA_i = [W_i0 (p 0..63) ; W_i1 (p 64..127)]   DA = [deformed_0 ; deformed_1]
      LB_i = [W_i2          ; wt_i            ]   DB = [deformed_2 ; ones     ]
    (the ones block folds the translation term into the same contraction)
  - Stacked trig operands (U=[sc;cc], UX=[cc;sc], SBX=[sb;-sb], CBX=[cb;-cb])
    let each rotation-row build be a single full-lane DVE op:
      LA0 = WS*(CBX*UX), LA1 = WCA*U + WSA*V, LA2 = WSA*U - WCA*V, V = SBX*UX
  - PSUM is DMA'd straight to DRAM as 6 [128, VC] planes; the host gather
    transposes to the reference [BS, V, 3] layout.
"""

import numpy as np
from contextlib import ExitStack

import concourse.bass as bass
import concourse.tile as tile
from concourse import bacc, mybir
from concourse.bass_utils import run_bass_kernel_spmd

B, S, P, V = 16, 16, 64, 2562
BS = B * S              # 256
N_CORES = 8
VPAD = 2576             # multiple of 8; per-core N kept even
VC = VPAD // N_CORES    # 322 vertices per core

F32 = mybir.dt.float32
F16 = mybir.dt.float16
AF = mybir.ActivationFunctionType
ALU = mybir.AluOpType
HALF_PI = float(np.pi / 2)


def _build_kernel():
    nc = bacc.Bacc("TRN2", target_bir_lowering=False, debug=False)

    ang_d = nc.dram_tensor("ang", [128, 3 * BS], F32, kind="ExternalInput").ap()
    wraw_d = nc.dram_tensor("wraw", [128, BS], F16, kind="ExternalInput").ap()
    scl_d = nc.dram_tensor("scl", [128, BS], F16, kind="ExternalInput").ap()
    trn_d = nc.dram_tensor("trn", [64, 3 * BS], F16, kind="ExternalInput").ap()
    offt_d = nc.dram_tensor("offt", [192, VC], F16, kind="ExternalInput").ap()
    bset_d = nc.dram_tensor("bset", [192, VC], F16, kind="ExternalInput").ap()
    ones_d = nc.dram_tensor("ones", [64, VC], F16, kind="ExternalInput").ap()
    out_d = nc.dram_tensor("out", [6, 128, VC], F16, kind="ExternalOutput").ap()

    lo = slice(0, 64)
    hi = slice(64, 128)
    c_a = slice(0, BS)
    c_b = slice(BS, 2 * BS)
    c_c = slice(2 * BS, 3 * BS)

    with tile.TileContext(nc) as tc, ExitStack() as ctx:
        pool = ctx.enter_context(tc.tile_pool(name="work", bufs=1))
        psum = ctx.enter_context(tc.tile_pool(name="psum", bufs=6, space="PSUM"))

        # ---- input tiles ----
        ang = pool.tile([128, 3 * BS], F32, tag="ang")
        wraw = pool.tile([128, BS], F16, tag="wraw")
        scl = pool.tile([128, BS], F16, tag="scl")
        trn = pool.tile([128, 3 * BS], F16, tag="trn")   # data in partitions 64:128
        da = pool.tile([128, VC], F16, tag="da")
        db = pool.tile([128, VC], F16, tag="db")
        offa = pool.tile([128, VC], F16, tag="offa")
        offb = pool.tile([64, VC], F16, tag="offb")
        bsa = pool.tile([128, VC], F16, tag="bsa")
        bsb = pool.tile([64, VC], F16, tag="bsb")

        nc.sync.dma_start(out=ang[:], in_=ang_d[:])
        nc.sync.dma_start(out=wraw[:], in_=wraw_d[:])
        nc.sync.dma_start(out=scl[:], in_=scl_d[:])
        nc.sync.dma_start(out=trn[64:128, :], in_=trn_d[:])
        nc.sync.dma_start(out=offa[:], in_=offt_d[0:128, :])
        nc.sync.dma_start(out=offb[:], in_=offt_d[128:192, :])
        nc.sync.dma_start(out=bsa[:], in_=bset_d[0:128, :])
        nc.sync.dma_start(out=bsb[:], in_=bset_d[128:192, :])
        nc.sync.dma_start(out=db[64:128, :], in_=ones_d[:])

        # ---- deformed (rhs) ----
        nc.vector.tensor_add(da[:], offa[:], bsa[:])          # deformed j=0,1
        nc.vector.tensor_add(db[0:64, :], offb[:], bsb[:])    # deformed j=2

        # ---- trig (ACT) ----
        # Sin spline domain is [-pi, pi] but |angle| can reach ~4.8. Half-angle
        # identity keeps every Sin argument in-domain (|x/2| < pi):
        #   sh = sin(x/2), ch = cos(x/2) = sin(pi/2 - |x|/2)
        #   sin(x) = 2*sh*ch,  cos(x) = 1 - 2*sh^2
        sh = pool.tile([128, 3 * BS], F16, tag="sh")
        ch = pool.tile([128, 3 * BS], F16, tag="ch")
        axa = pool.tile([128, 3 * BS], F32, tag="axa")
        halfpi = pool.tile([128, 1], F32, tag="halfpi")
        nc.vector.memset(halfpi[:], HALF_PI)
        nc.scalar.activation(axa[:], ang[:], AF.Abs)
        nc.scalar.activation(sh[:], ang[:], AF.Sin, scale=0.5)
        nc.scalar.activation(ch[:], axa[:], AF.Sin, bias=halfpi[:], scale=-0.5)

        sinall = pool.tile([128, 3 * BS], F16, tag="sinall")
        cosall = pool.tile([128, 3 * BS], F16, tag="cosall")
        sh2 = pool.tile([128, 3 * BS], F16, tag="sh2")
        nc.vector.scalar_tensor_tensor(
            sinall[:], sh[:], 2.0, ch[:], op0=ALU.mult, op1=ALU.mult)
        nc.vector.tensor_mul(sh2[:], sh[:], sh[:])
        nc.vector.tensor_scalar(cosall[:], sh2[:], -2.0, 1.0,
                                op0=ALU.mult, op1=ALU.add)

        # ---- stacked trig operands ----
        u = pool.tile([128, BS], F16, tag="u")        # [sc ; cc]
        ux = pool.tile([128, BS], F16, tag="ux")      # [cc ; sc]
        sbx = pool.tile([128, BS], F16, tag="sbx")    # [sb ; -sb]
        cbx = pool.tile([128, BS], F16, tag="cbx")    # [cb ; -cb]
        nc.vector.tensor_copy(u[lo, :], sinall[lo, c_c])
        nc.vector.tensor_copy(u[hi, :], cosall[hi, c_c])
        nc.vector.tensor_copy(ux[lo, :], cosall[lo, c_c])
        nc.vector.tensor_copy(ux[hi, :], sinall[hi, c_c])
        nc.vector.tensor_copy(sbx[lo, :], sinall[lo, c_b])
        nc.vector.tensor_scalar_mul(sbx[hi, :], sinall[hi, c_b], -1.0)
        nc.vector.tensor_copy(cbx[lo, :], cosall[lo, c_b])
        nc.vector.tensor_scalar_mul(cbx[hi, :], cosall[hi, c_b], -1.0)

        # ---- weight products ----
        ws = pool.tile([128, BS], F16, tag="ws")
        wca = pool.tile([128, BS], F16, tag="wca")
        wsa = pool.tile([128, BS], F16, tag="wsa")
        p1 = pool.tile([128, BS], F16, tag="p1")      # [cbcc ; -cbsc]
        v = pool.tile([128, BS], F16, tag="v")        # [sbcc ; -sbsc]
        la = [pool.tile([128, BS], F16, name=f"la{i}", tag=f"la{i}") for i in range(3)]
        lb = [pool.tile([128, BS], F16, name=f"lb{i}", tag=f"lb{i}") for i in range(3)]
        ta = pool.tile([128, BS], F16, tag="ta")
        tb = pool.tile([128, BS], F16, tag="tb")
        tc_ = pool.tile([128, BS], F16, tag="tc_")
        td = pool.tile([128, BS], F16, tag="td")

        nc.vector.tensor_mul(ws[:], wraw[:], scl[:])
        nc.vector.tensor_mul(wca[:], ws[:], cosall[:, c_a])
        nc.vector.tensor_mul(wsa[:], ws[:], sinall[:, c_a])

        # i=0 row first so PE can start early
        nc.vector.tensor_mul(p1[:], cbx[:], ux[:])
        nc.vector.tensor_mul(la[0][:], ws[:], p1[:])
        nc.vector.tensor_mul(lb[0][lo, :], ws[lo, :], sbx[lo, :])
        nc.gpsimd.tensor_mul(lb[0][hi, :], wraw[hi, :], trn[hi, c_a])

        # i=1 row
        nc.vector.tensor_mul(v[:], sbx[:], ux[:])
        nc.vector.tensor_mul(ta[:], wca[:], u[:])
        nc.vector.tensor_mul(tb[:], wsa[:], v[:])
        nc.vector.tensor_add(la[1][:], ta[:], tb[:])
        nc.vector.scalar_tensor_tensor(
            lb[1][lo, :], cbx[lo, :], -1.0, wsa[lo, :], op0=ALU.mult, op1=ALU.mult)
        nc.gpsimd.tensor_mul(lb[1][hi, :], wraw[hi, :], trn[hi, c_b])

        # i=2 row
        nc.vector.tensor_mul(tc_[:], wsa[:], u[:])
        nc.gpsimd.tensor_mul(td[:], wca[:], v[:])
        nc.vector.tensor_sub(la[2][:], tc_[:], td[:])
        nc.gpsimd.tensor_mul(lb[2][lo, :], wca[lo, :], cbx[lo, :])
        nc.gpsimd.tensor_mul(lb[2][hi, :], wraw[hi, :], trn[hi, c_c])

        # ---- matmuls + drain + output ----
        for i in range(3):
            for h in range(2):
                ms = slice(h * 128, (h + 1) * 128)
                ps = psum.tile([128, VC], F32)
                nc.tensor.matmul(ps[:], la[i][:, ms], da[:], start=True, stop=False)
                nc.tensor.matmul(ps[:], lb[i][:, ms], db[:], start=False, stop=True)
                osb = pool.tile([128, VC], F16, name=f"osb{i}{h}", tag=f"osb{i}{h}")
                if h == 0:
                    nc.vector.tensor_copy(osb[:], ps[:])
                else:
                    nc.scalar.copy(osb[:], ps[:])
                nc.sync.dma_start(out=out_d[i * 2 + h], in_=osb[:])

    nc.compile()
    return nc


_NC_CACHE = None


def _get_nc():
    global _NC_CACHE
    if _NC_CACHE is None:
        _NC_CACHE = _build_kernel()
    return _NC_CACHE


def _prep_inputs(scales, transforms, prototype_weights, prototype_offsets, base_verts):
    """Host-side shard/layout prep (layout + dtype staging only)."""
    f = np.float32
    h = np.float16
    scl1 = np.asarray(scales, f).reshape(BS)
    tf = np.asarray(transforms, f).reshape(BS, P, 6)

    ang_h = np.ascontiguousarray(tf[:, :, 3:6].transpose(1, 2, 0).reshape(P, 3 * BS))
    ang = np.concatenate([ang_h, ang_h], axis=0).astype(f)        # [128, 768]
    w_h = np.asarray(prototype_weights, f).reshape(BS, P).T
    wraw = np.concatenate([w_h, w_h], axis=0).astype(h)           # [128, 256]
    scl = np.broadcast_to(scl1[None, :], (128, BS)).astype(h)
    trn = tf[:, :, 0:3].transpose(1, 2, 0).reshape(P, 3 * BS).astype(h)

    offp = np.zeros((P, VPAD, 3), f)
    offp[:, :V] = np.asarray(prototype_offsets, f)
    offt = offp.transpose(2, 0, 1).reshape(192, VPAD).astype(h)
    basep = np.zeros((VPAD, 3), f)
    basep[:V] = np.asarray(base_verts, f)
    bset = np.broadcast_to(
        basep.T[:, None, :], (3, P, VPAD)).reshape(192, VPAD).astype(h)

    in_maps = []
    for core in range(N_CORES):
        vs = slice(core * VC, (core + 1) * VC)
        in_maps.append({
            "ang": ang,
            "wraw": wraw,
            "scl": scl,
            "trn": np.ascontiguousarray(trn),
            "offt": np.ascontiguousarray(offt[:, vs]),
            "bset": np.ascontiguousarray(bset[:, vs]),
            "ones": np.ones((64, VC), h),
        })
    return in_maps


def kernel(scales, transforms, prototype_weights, prototype_offsets, base_verts):
    nc = _get_nc()
    in_maps = _prep_inputs(
        scales, transforms, prototype_weights, prototype_offsets, base_verts)
    res = run_bass_kernel_spmd(nc, in_maps, list(range(N_CORES)))
    full = np.empty((BS, VPAD, 3), np.float32)
    for c in range(N_CORES):
        planes = res.results[c]["out"]          # [6, 128, VC]: plane = i*2 + h
        vs = slice(c * VC, (c + 1) * VC)
        for i in range(3):
            for hh in range(2):
                full[hh * 128:(hh + 1) * 128, vs, i] = planes[i * 2 + hh]
    return np.ascontiguousarray(full[:, :V, :])


# revision 13
# speedup vs baseline: 1.6309x; 1.1097x over previous
"""Trainium2 Bass kernel for nn_MeshTransformer (8-core SPMD, V-sharded).

Computes, for each of BS=256 (b,s) pairs:
    out[bs, v, i] = sum_{p,j} ws[bs,p] * R[i,j](bs,p) * deformed[p,v,j]
                    + sum_p w[bs,p] * t[bs,p,i]
with R the XYZ-euler rotation, ws = w * scale, deformed = base + offsets.

Mapping:
  - Vertex dim V (2562, padded to 2576) is sharded 8 ways (322/core).
  - Each core computes all 256 weight matrices on-chip (trig on ACT via
    half/quarter-angle identities, products on DVE/GPSIMD in fp16) and
    contracts them against its deformed slice on the PE (fp16 matmuls,
    fp32 PSUM).
  - The Sin spline only covers [-pi, pi] while |angle| reaches ~4.8, so:
      sh = sin(x/2), q = sin(x/4)           (both in-domain, no range fixup)
      ch = cos(x/2) = 1 - 2 q^2
      sin(x) = 2 sh ch,   cos(x) = 1 - 2 sh^2
  - lhsT partition layout packs rotation column j in 64-partition blocks,
    paired with a stacked rhs:
      LA_i = [W_i0 (p 0..63) ; W_i1 (p 64..127)]   DA = [deformed_0 ; deformed_1]
      LB_i = [W_i2          ; wt_i            ]   DB = [deformed_2 ; ones     ]
    (the ones block folds the translation term into the same contraction)
  - Stacked trig operands (U=[sc;cc], UX=[cc;sc], SBX=[sb;-sb], CBX=[cb;-cb])
    let each rotation-row build be a single full-lane DVE op:
      LA0 = WS*(CBX*UX), LA1 = WCA*U + WSA*V, LA2 = WSA*U - WCA*V, V = SBX*UX
  - PSUM is drained to fp16 plane tiles and DMA'd as 6 [128, VC] planes;
    the host gather transposes to the reference [BS, V, 3] layout.
"""

import numpy as np
from contextlib import ExitStack

import concourse.bass as bass
import concourse.tile as tile
from concourse import bacc, mybir
from concourse.bass_utils import run_bass_kernel_spmd

B, S, P, V = 16, 16, 64, 2562
BS = B * S              # 256
N_CORES = 8
VPAD = 2576             # multiple of 8; per-core N kept even
VC = VPAD // N_CORES    # 322 vertices per core

F32 = mybir.dt.float32
F16 = mybir.dt.float16
AF = mybir.ActivationFunctionType
ALU = mybir.AluOpType


def _build_kernel():
    nc = bacc.Bacc("TRN2", target_bir_lowering=False, debug=False)

    # ang | wraw | scl | trn packed into one [128, 2048] fp16 input
    wst_d = nc.dram_tensor("wst", [128, 2048], F16, kind="ExternalInput").ap()
    dmat_d = nc.dram_tensor("dmat", [192, 2 * VC], F16, kind="ExternalInput").ap()
    ones_d = nc.dram_tensor("ones", [64, VC], F16, kind="ExternalInput").ap()
    out_d = nc.dram_tensor("out", [6, 128, VC], F16, kind="ExternalOutput").ap()

    lo = slice(0, 64)
    hi = slice(64, 128)
    c_a = slice(0, BS)
    c_b = slice(BS, 2 * BS)
    c_c = slice(2 * BS, 3 * BS)

    with tile.TileContext(nc) as tc, ExitStack() as ctx:
        pool = ctx.enter_context(tc.tile_pool(name="work", bufs=1))
        psum = ctx.enter_context(tc.tile_pool(name="psum", bufs=6, space="PSUM"))

        # preload the ACT Sin table set before any data arrives
        dummy = pool.tile([128, 1], F16, tag="dummy")
        dummy2 = pool.tile([128, 1], F16, tag="dummy2")
        nc.vector.memset(dummy[:], 0.25)
        nc.scalar.activation(dummy2[:], dummy[:], AF.Sin)

        # ---- input tiles ----
        ang = pool.tile([128, 3 * BS], F16, tag="ang")
        wst = pool.tile([128, 1280], F16, tag="wst")
        dta = pool.tile([128, 2 * VC], F16, tag="dta")
        dtb = pool.tile([64, 2 * VC], F16, tag="dtb")
        da = pool.tile([128, VC], F16, tag="da")
        db = pool.tile([128, VC], F16, tag="db")

        nc.sync.dma_start(out=ang[:], in_=wst_d[:, 0:768])
        nc.sync.dma_start(out=wst[:], in_=wst_d[:, 768:2048])
        nc.sync.dma_start(out=dta[:], in_=dmat_d[0:128, :])
        nc.sync.dma_start(out=dtb[:], in_=dmat_d[128:192, :])
        nc.sync.dma_start(out=db[64:128, :], in_=ones_d[:])
        wraw = wst[:, 0:BS]
        scl = wst[:, BS:2 * BS]
        trn = wst[:, 2 * BS:5 * BS]

        # ---- deformed (rhs) ----
        nc.vector.tensor_add(da[:], dta[:, 0:VC], dta[:, VC:2 * VC])
        nc.vector.tensor_add(db[0:64, :], dtb[:, 0:VC], dtb[:, VC:2 * VC])

        # ---- trig (ACT + DVE) ----
        sh = pool.tile([128, 3 * BS], F16, tag="sh")
        q = pool.tile([128, 3 * BS], F16, tag="q")
        q2 = pool.tile([128, 3 * BS], F16, tag="q2")
        ch = pool.tile([128, 3 * BS], F16, tag="ch")
        nc.scalar.activation(sh[:], ang[:], AF.Sin, scale=0.5)
        nc.scalar.activation(q[:], ang[:], AF.Sin, scale=0.25)
        nc.vector.tensor_mul(q2[:], q[:], q[:])
        nc.vector.tensor_scalar(ch[:], q2[:], -2.0, 1.0, op0=ALU.mult, op1=ALU.add)

        sinall = pool.tile([128, 3 * BS], F16, tag="sinall")
        cosall = pool.tile([128, 3 * BS], F16, tag="cosall")
        sh2 = pool.tile([128, 3 * BS], F16, tag="sh2")
        nc.vector.scalar_tensor_tensor(
            sinall[:], sh[:], 2.0, ch[:], op0=ALU.mult, op1=ALU.mult)
        nc.vector.tensor_mul(sh2[:], sh[:], sh[:])
        nc.vector.tensor_scalar(cosall[:], sh2[:], -2.0, 1.0,
                                op0=ALU.mult, op1=ALU.add)

        # ---- stacked trig operands ----
        u = pool.tile([128, BS], F16, tag="u")        # [sc ; cc]
        ux = pool.tile([128, BS], F16, tag="ux")      # [cc ; sc]
        sbx = pool.tile([128, BS], F16, tag="sbx")    # [sb ; -sb]
        cbx = pool.tile([128, BS], F16, tag="cbx")    # [cb ; -cb]
        nc.vector.tensor_copy(u[lo, :], sinall[lo, c_c])
        nc.scalar.copy(u[hi, :], cosall[hi, c_c])
        nc.vector.tensor_copy(ux[lo, :], cosall[lo, c_c])
        nc.vector.tensor_copy(ux[hi, :], sinall[hi, c_c])
        nc.vector.tensor_copy(sbx[lo, :], sinall[lo, c_b])
        nc.scalar.mul(sbx[hi, :], sinall[hi, c_b], -1.0)
        nc.vector.tensor_copy(cbx[lo, :], cosall[lo, c_b])
        nc.scalar.mul(cbx[hi, :], cosall[hi, c_b], -1.0)

        # ---- weight products ----
        ws = pool.tile([128, BS], F16, tag="ws")
        wca = pool.tile([128, BS], F16, tag="wca")
        wsa = pool.tile([128, BS], F16, tag="wsa")
        p1 = pool.tile([128, BS], F16, tag="p1")      # [cbcc ; -cbsc]
        v = pool.tile([128, BS], F16, tag="v")        # [sbcc ; -sbsc]
        la = [pool.tile([128, BS], F16, name=f"la{i}", tag=f"la{i}") for i in range(3)]
        lb = [pool.tile([128, BS], F16, name=f"lb{i}", tag=f"lb{i}") for i in range(3)]
        ta = pool.tile([128, BS], F16, tag="ta")
        tb = pool.tile([128, BS], F16, tag="tb")
        tc_ = pool.tile([128, BS], F16, tag="tc_")
        td = pool.tile([128, BS], F16, tag="td")

        nc.vector.tensor_mul(ws[:], wraw, scl)
        nc.vector.tensor_mul(wca[:], ws[:], cosall[:, c_a])
        nc.vector.tensor_mul(wsa[:], ws[:], sinall[:, c_a])

        # i=0 row first so PE can start early
        nc.vector.tensor_mul(p1[:], cbx[:], ux[:])
        nc.vector.tensor_mul(la[0][:], ws[:], p1[:])
        nc.vector.tensor_mul(lb[0][lo, :], ws[lo, :], sbx[lo, :])
        nc.gpsimd.tensor_mul(lb[0][hi, :], wraw[hi, :], trn[hi, 0:BS])

        # i=1 row
        nc.vector.tensor_mul(v[:], sbx[:], ux[:])
        nc.vector.tensor_mul(ta[:], wca[:], u[:])
        nc.vector.tensor_mul(tb[:], wsa[:], v[:])
        nc.vector.tensor_add(la[1][:], ta[:], tb[:])
        nc.vector.scalar_tensor_tensor(
            lb[1][lo, :], cbx[lo, :], -1.0, wsa[lo, :], op0=ALU.mult, op1=ALU.mult)
        nc.gpsimd.tensor_mul(lb[1][hi, :], wraw[hi, :], trn[hi, BS:2 * BS])

        # i=2 row
        nc.vector.tensor_mul(tc_[:], wsa[:], u[:])
        nc.gpsimd.tensor_mul(td[:], wca[:], v[:])
        nc.vector.tensor_sub(la[2][:], tc_[:], td[:])
        nc.gpsimd.tensor_mul(lb[2][lo, :], wca[lo, :], cbx[lo, :])
        nc.gpsimd.tensor_mul(lb[2][hi, :], wraw[hi, :], trn[hi, 2 * BS:3 * BS])

        # ---- matmuls + drain + output ----
        for i in range(3):
            for h in range(2):
                ms = slice(h * 128, (h + 1) * 128)
                ps = psum.tile([128, VC], F32)
                nc.tensor.matmul(ps[:], la[i][:, ms], da[:], start=True, stop=False)
                nc.tensor.matmul(ps[:], lb[i][:, ms], db[:], start=False, stop=True)
                osb = pool.tile([128, VC], F16, name=f"osb{i}{h}", tag=f"osb{i}{h}")
                if h == 0:
                    nc.vector.tensor_copy(osb[:], ps[:])
                else:
                    nc.scalar.copy(osb[:], ps[:])
                nc.sync.dma_start(out=out_d[i * 2 + h], in_=osb[:])

    nc.compile()
    return nc


_NC_CACHE = None


def _get_nc():
    global _NC_CACHE
    if _NC_CACHE is None:
        _NC_CACHE = _build_kernel()
    return _NC_CACHE


def _prep_inputs(scales, transforms, prototype_weights, prototype_offsets, base_verts):
    """Host-side shard/layout prep (layout + dtype staging only)."""
    f = np.float32
    hh = np.float16
    scl1 = np.asarray(scales, f).reshape(BS)
    tf = np.asarray(transforms, f).reshape(BS, P, 6)

    ang_h = tf[:, :, 3:6].transpose(1, 2, 0).reshape(P, 3 * BS)
    ang = np.concatenate([ang_h, ang_h], axis=0)                  # [128, 768]
    w_h = np.asarray(prototype_weights, f).reshape(BS, P).T
    wraw = np.concatenate([w_h, w_h], axis=0)                     # [128, 256]
    scl = np.broadcast_to(scl1[None, :], (128, BS))
    trn_h = tf[:, :, 0:3].transpose(1, 2, 0).reshape(P, 3 * BS)
    trn = np.concatenate([trn_h, trn_h], axis=0)                  # [128, 768]
    wst = np.concatenate([ang, wraw, scl, trn], axis=1).astype(hh)  # [128, 2048]

    offp = np.zeros((P, VPAD, 3), f)
    offp[:, :V] = np.asarray(prototype_offsets, f)
    offt = offp.transpose(2, 0, 1).reshape(192, VPAD)
    basep = np.zeros((VPAD, 3), f)
    basep[:V] = np.asarray(base_verts, f)
    bset = np.broadcast_to(basep.T[:, None, :], (3, P, VPAD)).reshape(192, VPAD)

    ones = np.ones((64, VC), hh)
    in_maps = []
    for core in range(N_CORES):
        vs = slice(core * VC, (core + 1) * VC)
        dmat = np.concatenate([offt[:, vs], bset[:, vs]], axis=1).astype(hh)
        in_maps.append({"wst": wst, "dmat": dmat, "ones": ones})
    return in_maps


def kernel(scales, transforms, prototype_weights, prototype_offsets, base_verts):
    nc = _get_nc()
    in_maps = _prep_inputs(
        scales, transforms, prototype_weights, prototype_offsets, base_verts)
    res = run_bass_kernel_spmd(nc, in_maps, list(range(N_CORES)))
    full = np.empty((BS, VPAD, 3), np.float32)
    for c in range(N_CORES):
        planes = res.results[c]["out"].astype(np.float32)   # [6, 128, VC]
        vs = slice(c * VC, (c + 1) * VC)
        for i in range(3):
            for h in range(2):
                full[h * 128:(h + 1) * 128, vs, i] = planes[i * 2 + h]
    return np.ascontiguousarray(full[:, :V, :])


# revision 15
# speedup vs baseline: 1.9365x; 1.1873x over previous
"""Trainium2 Bass kernel for nn_MeshTransformer (8-core SPMD, V-sharded).

Computes, for each of BS=256 (b,s) pairs:
    out[bs, v, i] = sum_{p,j} ws[bs,p] * R[i,j](bs,p) * deformed[p,v,j]
                    + sum_p w[bs,p] * t[bs,p,i]
with R the XYZ-euler rotation, ws = w * scale, deformed = base + offsets.

Mapping:
  - Vertex dim V (2562, padded to 2576) is sharded 8 ways (322/core).
  - Each core computes all 256 weight matrices on-chip and contracts them
    against its deformed slice on the PE (fp16 matmuls, fp32 PSUM).
  - The host ships six 256-col angle blocks, each range-folded to [-pi, pi)
    (Sin spline domain) and pre-shifted so that ONE ACT Sin op yields every
    needed trig operand, including the stacked/negated forms, as views:
      S = sin(ang6) = [ sa | ca | (sc;cc) | (cc;sc) | (sb;-sb) | (cb;-cb) ]
    (cos(x) = sin(pi/2 - x); the two 64-partition halves of a block hold
    different shifts, matching the lhsT partition packing below.)
  - lhsT partition layout packs rotation column j in 64-partition blocks,
    paired with a stacked rhs:
      LA_i = [W_i0 (p 0..63) ; W_i1 (p 64..127)]   DA = [deformed_0 ; deformed_1]
      LB_i = [W_i2          ; wt_i            ]   DB = [deformed_2 ; ones     ]
    (the ones block folds the translation term into the same contraction),
    so each rotation-row build is a single full-lane DVE op:
      LA0 = WS*(CBX*UX), LA1 = WCA*U + WSA*V, LA2 = WSA*U - WCA*V, V = SBX*UX
  - PSUM is drained to fp16 plane tiles and DMA'd as 6 [128, VC] planes;
    the host gather transposes to the reference [BS, V, 3] layout.
"""

import numpy as np
from contextlib import ExitStack

import concourse.bass as bass
import concourse.tile as tile
from concourse import bacc, mybir
from concourse.bass_utils import run_bass_kernel_spmd

B, S, P, V = 16, 16, 64, 2562
BS = B * S              # 256
N_CORES = 8
VPAD = 2576             # multiple of 8; per-core N kept even
VC = VPAD // N_CORES    # 322 vertices per core

F32 = mybir.dt.float32
F16 = mybir.dt.float16
AF = mybir.ActivationFunctionType
ALU = mybir.AluOpType


def _build_kernel():
    nc = bacc.Bacc("TRN2", target_bir_lowering=False, debug=False)

    ang_d = nc.dram_tensor("ang6", [128, 1536], F16, kind="ExternalInput").ap()
    # wraw | scl | trn | offtA | bsetA | offtB/bsetB (rows 0:64)
    rest_d = nc.dram_tensor("rest", [128, 1280 + 4 * VC], F16,
                            kind="ExternalInput").ap()
    out_d = nc.dram_tensor("out", [6, 128, VC], F16, kind="ExternalOutput").ap()

    lo = slice(0, 64)
    hi = slice(64, 128)

    with tile.TileContext(nc) as tc, ExitStack() as ctx:
        pool = ctx.enter_context(tc.tile_pool(name="work", bufs=1))
        psum = ctx.enter_context(tc.tile_pool(name="psum", bufs=6, space="PSUM"))

        # preload the ACT Sin table set while the inputs are still in flight
        dummy = pool.tile([128, 1], F16, tag="dummy")
        dummy2 = pool.tile([128, 1], F16, tag="dummy2")
        nc.vector.memset(dummy[:], 0.25)
        nc.scalar.activation(dummy2[:], dummy[:], AF.Sin)

        # ---- input tiles ----
        ang = pool.tile([128, 1536], F16, tag="ang")
        rest = pool.tile([128, 1280 + 4 * VC], F16, tag="rest")
        nc.sync.dma_start(out=ang[:], in_=ang_d[:])
        nc.sync.dma_start(out=rest[:], in_=rest_d[:])
        wraw = rest[:, 0:BS]
        scl = rest[:, BS:2 * BS]
        trn = rest[:, 2 * BS:5 * BS]
        dta = rest[:, 1280:1280 + 2 * VC]                 # offtA | bsetA
        dtb = rest[0:64, 1280 + 2 * VC:1280 + 4 * VC]     # offtB | bsetB (rows 0:64)

        # ---- deformed (rhs) ----
        da = pool.tile([128, VC], F16, tag="da")
        db = pool.tile([128, VC], F16, tag="db")
        nc.vector.tensor_add(da[:], dta[:, 0:VC], dta[:, VC:2 * VC])
        nc.vector.tensor_add(db[0:64, :], dtb[:, 0:VC], dtb[:, VC:2 * VC])
        nc.vector.memset(db[64:128, :], 1.0)         # translation ones block

        # ---- trig: one Sin over all pre-folded blocks ----
        sall = pool.tile([128, 1536], F16, tag="sall")
        nc.scalar.activation(sall[:], ang[:], AF.Sin)
        sa = sall[:, 0:256]
        ca = sall[:, 256:512]
        u = sall[:, 512:768]        # [sc ; cc]
        ux = sall[:, 768:1024]      # [cc ; sc]
        sbx = sall[:, 1024:1280]    # [sb ; -sb]
        cbx = sall[:, 1280:1536]    # [cb ; -cb]

        # ---- weight products (fp16, full-lane) ----
        ws = pool.tile([128, BS], F16, tag="ws")
        wca = pool.tile([128, BS], F16, tag="wca")
        wsa = pool.tile([128, BS], F16, tag="wsa")
        p1 = pool.tile([128, BS], F16, tag="p1")      # [cbcc ; -cbsc]
        v = pool.tile([128, BS], F16, tag="v")        # [sbcc ; -sbsc]
        la = [pool.tile([128, BS], F16, name=f"la{i}", tag=f"la{i}") for i in range(3)]
        lb = [pool.tile([128, BS], F16, name=f"lb{i}", tag=f"lb{i}") for i in range(3)]
        ta = pool.tile([128, BS], F16, tag="ta")
        tb = pool.tile([128, BS], F16, tag="tb")
        tc_ = pool.tile([128, BS], F16, tag="tc_")
        td = pool.tile([128, BS], F16, tag="td")

        # translation weights: no trig dependency, run during the Sin op
        nc.gpsimd.tensor_mul(lb[0][hi, :], wraw[hi, :], trn[hi, 0:BS])
        nc.gpsimd.tensor_mul(lb[1][hi, :], wraw[hi, :], trn[hi, BS:2 * BS])
        nc.gpsimd.tensor_mul(lb[2][hi, :], wraw[hi, :], trn[hi, 2 * BS:3 * BS])

        nc.vector.tensor_mul(ws[:], wraw, scl)
        nc.vector.tensor_mul(wca[:], ws[:], ca)
        nc.vector.tensor_mul(wsa[:], ws[:], sa)

        # i=0 row first so PE can start early
        nc.vector.tensor_mul(p1[:], cbx, ux)
        nc.vector.tensor_mul(la[0][:], ws[:], p1[:])
        nc.vector.tensor_mul(lb[0][lo, :], ws[lo, :], sbx[lo, :])

        # i=1 row
        nc.vector.tensor_mul(v[:], sbx, ux)
        nc.vector.tensor_mul(ta[:], wca[:], u)
        nc.vector.tensor_mul(tb[:], wsa[:], v[:])
        nc.vector.tensor_add(la[1][:], ta[:], tb[:])
        nc.vector.scalar_tensor_tensor(
            lb[1][lo, :], cbx[lo, :], -1.0, wsa[lo, :], op0=ALU.mult, op1=ALU.mult)

        # i=2 row
        nc.vector.tensor_mul(tc_[:], wsa[:], u)
        nc.vector.tensor_mul(td[:], wca[:], v[:])
        nc.vector.tensor_sub(la[2][:], tc_[:], td[:])
        nc.vector.tensor_mul(lb[2][lo, :], wca[lo, :], cbx[lo, :])

        # ---- matmuls + drain + output ----
        for i in range(3):
            for h in range(2):
                ms = slice(h * 128, (h + 1) * 128)
                ps = psum.tile([128, VC], F32)
                nc.tensor.matmul(ps[:], la[i][:, ms], da[:], start=True, stop=False)
                nc.tensor.matmul(ps[:], lb[i][:, ms], db[:], start=False, stop=True)
                osb = pool.tile([128, VC], F16, name=f"osb{i}{h}", tag=f"osb{i}{h}")
                if i == 2 and h == 0:
                    nc.vector.tensor_copy(osb[:], ps[:])
                else:
                    nc.scalar.copy(osb[:], ps[:])
                nc.sync.dma_start(out=out_d[i * 2 + h], in_=osb[:])

    nc.compile()
    return nc


_NC_CACHE = None


def _get_nc():
    global _NC_CACHE
    if _NC_CACHE is None:
        _NC_CACHE = _build_kernel()
    return _NC_CACHE


def _fold(x):
    """Range-fold to [-pi, pi) (Sin spline domain)."""
    return np.mod(x + np.pi, 2 * np.pi) - np.pi


def _prep_inputs(scales, transforms, prototype_weights, prototype_offsets, base_verts):
    """Host-side shard/layout prep (layout, dup, angle folding/shifting)."""
    f = np.float64
    hh = np.float16
    scl1 = np.asarray(scales, np.float32).reshape(BS)
    tf = np.asarray(transforms, np.float32).reshape(BS, P, 6)

    a = tf[:, :, 3].T.astype(f)   # [p, bs]
    b = tf[:, :, 4].T.astype(f)
    c = tf[:, :, 5].T.astype(f)
    P2 = np.pi / 2

    def blk(lov, hiv):
        return np.concatenate([_fold(lov), _fold(hiv)], axis=0)   # [128, bs]

    ang6 = np.concatenate([
        blk(a, a),              # sa
        blk(P2 - a, P2 - a),    # ca
        blk(c, P2 - c),         # [sc ; cc]
        blk(P2 - c, c),         # [cc ; sc]
        blk(b, -b),             # [sb ; -sb]
        blk(P2 - b, b - P2),    # [cb ; -cb]
    ], axis=1).astype(hh)                                         # [128, 1536]

    w_h = np.asarray(prototype_weights, np.float32).reshape(BS, P).T
    wraw = np.concatenate([w_h, w_h], axis=0)                     # [128, 256]
    scl = np.broadcast_to(scl1[None, :], (128, BS))
    trn_h = tf[:, :, 0:3].transpose(1, 2, 0).reshape(P, 3 * BS)
    trn = np.concatenate([trn_h, trn_h], axis=0)                  # [128, 768]

    offp = np.zeros((P, VPAD, 3), np.float32)
    offp[:, :V] = np.asarray(prototype_offsets, np.float32)
    offt = offp.transpose(2, 0, 1).reshape(192, VPAD)
    basep = np.zeros((VPAD, 3), np.float32)
    basep[:V] = np.asarray(base_verts, np.float32)
    bset = np.broadcast_to(basep.T[:, None, :], (3, P, VPAD)).reshape(192, VPAD)

    in_maps = []
    for core in range(N_CORES):
        vs = slice(core * VC, (core + 1) * VC)
        oA, bA = offt[0:128, vs], bset[0:128, vs]
        oB, bB = offt[128:192, vs], bset[128:192, vs]
        dB = np.zeros((128, 2 * VC), np.float32)
        dB[0:64, 0:VC] = oB
        dB[0:64, VC:2 * VC] = bB
        # layout: [0:256 wraw][256:512 scl][512:1280 trn]
        #         [1280:+VC offtA][+VC:+2VC bsetA][+2VC:+4VC offtB|bsetB (rows 0:64)]
        rest = np.concatenate([wraw, scl, trn, oA, bA, dB], axis=1)
        in_maps.append({"ang6": ang6, "rest": rest.astype(hh)})
    return in_maps


def kernel(scales, transforms, prototype_weights, prototype_offsets, base_verts):
    nc = _get_nc()
    in_maps = _prep_inputs(
        scales, transforms, prototype_weights, prototype_offsets, base_verts)
    res = run_bass_kernel_spmd(nc, in_maps, list(range(N_CORES)))
    full = np.empty((BS, VPAD, 3), np.float32)
    for c in range(N_CORES):
        planes = res.results[c]["out"].astype(np.float32)   # [6, 128, VC]
        vs = slice(c * VC, (c + 1) * VC)
        for i in range(3):
            for h in range(2):
                full[h * 128:(h + 1) * 128, vs, i] = planes[i * 2 + h]
    return np.ascontiguousarray(full[:, :V, :])
